# revision 1
# baseline (speedup 1.0000x reference)
"""Multi-head self-attention (B=4, S=2048, D=1024, H=16) on 8 TRN2 NeuronCores.

Sharding: data-parallel over batch x tensor-parallel over heads (Megatron
column-split of w_qkv, row-split of w_out). Core c computes batch c//2 with
heads (c%2)*8..(c%2)*8+8 and produces a partial [S, D] output; the host sums
the two partials per batch and adds the bias.

Per-core kernel (single Tile program, fp32r matmuls ~3e-4 rel err):
  - x is PE-transposed to xT (d-major) and staged to DRAM; v = x @ wv kept
    seq-major in SBUF with a ones column per head (softmax denominators).
  - qT/kT computed feat-major per head-pair [128, S], projected just-in-time
    and overlapped with the previous pair's (ACT-bound) attention.
  - per (pair, q-half, head): scoresT[k,q] = kT_chunk^T @ qT (K=64 one-shot),
    E = exp(scale*scoresT) on ACT, pv[65,q] += v~_chunk^T @ E with the ones
    row accumulating the denominator; PV runs one k-tile behind QK.
  - normalize: 1/denom (DVE) -> partition_broadcast (GPSIMD) -> multiply into
    the paired feat-major outT tile, deferred off the critical DVE queue.
  - y = sum_pairs outT^T @ wout at K=128, interleaved with the last pair.
"""

import numpy as np

from concourse import bass_utils



from contextlib import ExitStack

import concourse.bacc as bacc
import concourse.bass as bass
import concourse.mybir as mybir
import concourse.tile as tile
from concourse import masks

P = 128
HD = 64
HV = HD + 1
QCH = 512
F32 = mybir.dt.float32
F32R = mybir.dt.float32r
BF16 = mybir.dt.bfloat16
EXP = mybir.ActivationFunctionType.Exp


def build_attention(
    S: int,
    D: int,
    HN: int,
    DO: int,
    scale: float,
    dt_x=F32R,
    dt_qk=F32R,
    dt_e=F32R,
    dt_o=F32R,
) -> bacc.Bacc:
    F = HN * HD
    n_st = S // P
    n_dt = D // P
    n_ft = F // P
    n_ch = S // QCH
    n_kt = S // P
    n_no = DO // QCH
    QH = min(1024, S)
    n_qh = S // QH
    n_j = QH // QCH
    n_sti = QCH // P
    assert S % QCH == 0 and D % P == 0 and F % P == 0 and DO % QCH == 0

    small = mybir.dt.size(dt_x) == 2

    nc = bacc.Bacc("TRN2", target_bir_lowering=False, debug=False)

    x = nc.dram_tensor("x", [S, D], F32, kind="ExternalInput")
    wq = nc.dram_tensor("wq", [D, F], F32, kind="ExternalInput")
    wk = nc.dram_tensor("wk", [D, F], F32, kind="ExternalInput")
    wv = nc.dram_tensor("wv", [D, F], F32, kind="ExternalInput")
    wout = nc.dram_tensor("wout", [F, DO], F32, kind="ExternalInput")
    y = nc.dram_tensor("y", [S, DO], F32, kind="ExternalOutput")

    xT_dram = nc.dram_tensor("xT_scratch", [n_dt, P, S], dt_x)

    with tile.TileContext(nc) as tc, ExitStack() as top:  # noqa: PLR1702
        const_pool = top.enter_context(tc.tile_pool(name="const", bufs=1))
        ident = const_pool.tile([P, P], F32, tag="ident")
        masks.make_identity(nc, ident[:])
        ident_b = const_pool.tile([P, P], BF16, tag="identb")
        nc.vector.tensor_copy(ident_b[:], ident[:])
        ones_f32 = const_pool.tile([P, HD], F32, tag="ones_f32")
        nc.gpsimd.memset(ones_f32[:], 1.0)

        v_pool = top.enter_context(tc.tile_pool(name="vsb", bufs=1))
        v_sb = [
            v_pool.tile([P, HN * HV], dt_e, tag=f"v{st}", name=f"v_sb{st}")
            for st in range(n_st)
        ]
        for st in range(n_st):
            nc.vector.tensor_copy(
                v_sb[st][:].rearrange("p (h v) -> p h v", v=HV)[:, :, HD:].rearrange(
                    "p h one -> p (h one)"
                ),
                ones_f32[:, :HN],
            )

        outT_pool = top.enter_context(tc.tile_pool(name="outT", bufs=1))
        outP = [
            outT_pool.tile([P, S], dt_o, tag=f"o{ft}", name=f"outP{ft}")
            for ft in range(n_ft)
        ]

        wqk_pool = top.enter_context(tc.tile_pool(name="wqk", bufs=1))
        # one slab per (pair, q|k): [128, n_dt*128] with free = (db, col) —
        # a single DMA instead of n_dt small ones (SWDGE queue serializes)
        wq_s = [
            wqk_pool.tile([P, n_dt * P], dt_x, tag=f"wqs{ft}", name=f"wqs{ft}")
            for ft in range(n_ft)
        ]
        wk_s = [
            wqk_pool.tile([P, n_dt * P], dt_x, tag=f"wks{ft}", name=f"wks{ft}")
            for ft in range(n_ft)
        ]
        wq_t = [[wq_s[ft][:, db * P : (db + 1) * P] for ft in range(n_ft)] for db in range(n_dt)]
        wk_t = [[wk_s[ft][:, db * P : (db + 1) * P] for ft in range(n_ft)] for db in range(n_dt)]

        def load_w_pair(ft):
            nc.gpsimd.dma_start(
                wq_s[ft][:].rearrange("p (db c) -> p db c", c=P),
                wq[:, ft * P : (ft + 1) * P].rearrange("(db p) c -> p db c", p=P),
            )
            nc.gpsimd.dma_start(
                wk_s[ft][:].rearrange("p (db c) -> p db c", c=P),
                wk[:, ft * P : (ft + 1) * P].rearrange("(db p) c -> p db c", p=P),
            )

        pair_pool = top.enter_context(tc.tile_pool(name="pair", bufs=2))
        pair_tiles = {}

        def get_pair(ft):
            if ft not in pair_tiles:
                pair_tiles[ft] = (
                    pair_pool.tile([P, S], dt_qk, tag="qp", name=f"qTp{ft}"),
                    pair_pool.tile([P, S], dt_qk, tag="kp", name=f"kTp{ft}"),
                )
            return pair_tiles[ft]

        up_stack = ExitStack()
        up_on = top if small else up_stack
        wv_pool = up_on.enter_context(tc.tile_pool(name="wvp", bufs=1))
        wv_t = [
            wv_pool.tile([P, F], dt_x, tag=f"wv{db}", name=f"wv{db}")
            for db in range(n_dt)
        ]
        load_w_pair(0)
        for db in range(n_dt):
            nc.gpsimd.dma_start(wv_t[db][:], wv[db * P : (db + 1) * P, :])
        for ft in range(1, n_ft):
            load_w_pair(ft)

        # sbuf working pools; upfront pools (wv/xst/xTc) are scoped and
        # released before the main loop when dtypes are 4-byte (SBUF pressure)
        xst_pool = up_on.enter_context(
            tc.tile_pool(name="xst", bufs=(2 * n_sti) if small else max(4, n_sti))
        )
        xTc_pool = up_on.enter_context(
            tc.tile_pool(name="xTc", bufs=2)
        )
        ps_sc = top.enter_context(
            tc.tile_pool(name="ps_sc", bufs=3, space=bass.MemorySpace.PSUM)
        )
        ps_pv = top.enter_context(
            tc.tile_pool(name="ps_pv", bufs=1, space=bass.MemorySpace.PSUM)
        )
        e_pool = stg_pool = rc_pool = bcs_pool = xTs_pool = None
        xTs_stack = ExitStack()

        def make_main_pools():
            nonlocal e_pool, stg_pool, rc_pool, bcs_pool, xTs_pool
            e_pool = top.enter_context(
                tc.tile_pool(name="epool", bufs=4 if small else 2)
            )
            stg_pool = top.enter_context(
                tc.tile_pool(name="stgpool", bufs=3 if small else 2)
            )
            rc_pool = top.enter_context(tc.tile_pool(name="rcpool", bufs=2))
            bcs_pool = top.enter_context(tc.tile_pool(name="bcspool", bufs=2))
            xTs_pool = xTs_stack.enter_context(
                tc.tile_pool(name="xTs", bufs=3 if small else 2)
            )

        # ---------------- building blocks ----------------
        def upfront_chunk(ch, qTp0, kTp0, defer_v=False):
            xrows = []
            for sti in range(n_sti):
                st = ch * n_sti + sti
                xrow = xst_pool.tile([P, D], F32, tag="xrow", name=f"xrow{st}")
                nc.sync.dma_start(xrow[:], x[st * P : (st + 1) * P, :])
                if small:
                    xb = xst_pool.tile(
                        [P, D], dt_x, tag="xbf", bufs=2 * n_sti, name=f"xb{st}"
                    )
                    nc.vector.tensor_copy(xb[:], xrow[:])
                    xrows.append(xb)
                else:
                    xrows.append(xrow)
            xT = [
                xTc_pool.tile([P, QCH], dt_x, tag=f"xc{db}", name=f"xT{db}_{ch}")
                for db in range(n_dt)
            ]
            for db in range(n_dt):
                tp = ps_sc.tile(
                    [P, QCH], dt_x if small else F32, tag="sc", name=f"tr{ch}_{db}"
                )
                for sti in range(n_sti):
                    nc.tensor.transpose(
                        tp[:, sti * P : (sti + 1) * P],
                        xrows[sti][:, db * P : (db + 1) * P],
                        ident[:] if not small else ident_b[:],
                    )
                nc.vector.tensor_copy(xT[db][:], tp[:])
            for w_t, dstp in ((wq_t, qTp0), (wk_t, kTp0)):
                pp = ps_sc.tile([P, QCH], F32, tag="sc", name=f"pj0_{ch}")
                for db in range(n_dt):
                    nc.tensor.matmul(
                        pp[:],
                        w_t[db][0],
                        xT[db][:],
                        start=(db == 0),
                        stop=(db == n_dt - 1),
                    )
                nc.vector.tensor_copy(dstp[:, ch * QCH : (ch + 1) * QCH], pp[:])

            def v_item(sti):
                def run():
                    st = ch * n_sti + sti
                    pv_ps = ps_sc.tile([P, F], F32, tag="sc", name=f"pvp{st}")
                    for db in range(n_dt):
                        nc.tensor.matmul(
                            pv_ps[:],
                            xT[db][:, sti * P : (sti + 1) * P],
                            wv_t[db][:],
                            start=(db == 0),
                            stop=(db == n_dt - 1),
                        )
                    nc.vector.tensor_copy(
                        v_sb[st][:].rearrange("p (h v) -> p h v", v=HV)[:, :, :HD],
                        pv_ps[:].rearrange("p (h d) -> p h d", d=HD),
                    )

                return run

            def xout_item():
                def run():
                    for db in range(n_dt):
                        nc.sync.dma_start(
                            xT_dram[db, :, ch * QCH : (ch + 1) * QCH], xT[db][:]
                        )

                return run

            items = [v_item(sti) for sti in range(n_sti)] + [xout_item()]
            if defer_v:
                return items
            for it in items:
                it()
            return []

        def proj_work_items(ftn):
            items = []
            qTp, kTp = get_pair(ftn)
            xTs_tiles = {}

            def dma_item(ch):
                def run():
                    xTs_tiles[ch] = [
                        xTs_pool.tile(
                            [P, QCH], dt_x, tag=f"xs{db}", name=f"xs{db}_{ftn}_{ch}"
                        )
                        for db in range(n_dt)
                    ]
                    for db in range(n_dt):
                        nc.sync.dma_start(
                            xTs_tiles[ch][db][:],
                            xT_dram[db, :, ch * QCH : (ch + 1) * QCH],
                        )

                return run

            def mm_item(ch, w_t, dstp, which):
                def run():
                    pp = ps_sc.tile([P, QCH], F32, tag="sc", name=f"pj{which}{ftn}_{ch}")
                    for db in range(n_dt):
                        nc.tensor.matmul(
                            pp[:],
                            w_t[db][ftn],
                            xTs_tiles[ch][db][:],
                            start=(db == 0),
                            stop=(db == n_dt - 1),
                        )
                    nc.vector.tensor_copy(dstp[:, ch * QCH : (ch + 1) * QCH], pp[:])

                return run

            mm_items = []
            for ch in range(n_ch):
                items.append(dma_item(ch))
                mm_items.append(mm_item(ch, wq_t, qTp, "q"))
                mm_items.append(mm_item(ch, wk_t, kTp, "k"))
                # keep each DMA ~2 items ahead of its matmuls
                if ch >= 1:
                    items.append(mm_items.pop(0))
            items.extend(mm_items)
            return items

        def emit_pv(pvt, ets, kt, hA, hB):
            for parity, h in ((0, hA), (1, hB)):
                vt = v_sb[kt][:].rearrange("p (hh v) -> p hh v", v=HV)[:, h, :]
                for j in range(n_j):
                    nc.tensor.matmul(
                        pvt[parity][:, j * QCH : (j + 1) * QCH],
                        vt,
                        ets[parity][:, j * QCH : (j + 1) * QCH],
                        start=(kt == 0),
                        stop=(kt == n_kt - 1),
                    )

        class AttnQH:
            """Resumable emitter for one (pair, q-half, head-parity) attention
            pass. Single-parity passes keep PV in 2 PSUM banks, freeing a 3rd
            score slot so ACT never waits on the next QK."""

            def __init__(self, ft, qh, parity):
                self.ft, self.qh, self.parity = ft, qh, parity
                self.qTp, self.kTp = get_pair(ft)
                self.h = 2 * ft + parity
                self.q_base = qh * QH
                self.pv = ps_pv.tile(
                    [HV, QH], F32, tag="pv", name=f"pv{ft}_{qh}_{parity}"
                )
                self.prev = None

            def emit_kts(self, kts, sprinkle=None, stride=1):
                ft, qh, parity, q_base = self.ft, self.qh, self.parity, self.q_base
                sub = parity * HD
                for idx, kt in enumerate(kts):
                    if sprinkle and idx % stride == 0:
                        sprinkle.pop(0)()
                    sc = ps_sc.tile(
                        [P, QH], F32, tag="sc", name=f"sc{ft}{parity}{qh}{kt}"
                    )
                    for j in range(n_j):
                        q0 = q_base + j * QCH
                        nc.tensor.matmul(
                            sc[:, j * QCH : (j + 1) * QCH],
                            self.kTp[sub : sub + HD, kt * P : (kt + 1) * P],
                            self.qTp[sub : sub + HD, q0 : q0 + QCH],
                            start=True,
                            stop=True,
                        )
                    et = e_pool.tile(
                        [P, QH], dt_e, tag="et", name=f"e{ft}{parity}{qh}{kt}"
                    )
                    nc.scalar.activation(et[:], sc[:], EXP, scale=scale)
                    if self.prev is not None:
                        self._pv(self.prev)
                    self.prev = (kt, et)

            def _pv(self, prev):
                kt, et = prev
                vt = v_sb[kt][:].rearrange("p (hh v) -> p hh v", v=HV)[:, self.h, :]
                for j in range(n_j):
                    nc.tensor.matmul(
                        self.pv[:, j * QCH : (j + 1) * QCH],
                        vt,
                        et[:, j * QCH : (j + 1) * QCH],
                        start=(kt == 0),
                        stop=(kt == n_kt - 1),
                    )

            def finish_stage1(self):
                """Emit the last PV and copy pv to SBUF staging (frees the
                PSUM accumulator); normalization is deferred."""
                self._pv(self.prev)
                ft, qh, parity = self.ft, self.qh, self.parity
                self.stg = stg_pool.tile(
                    [HV, QH], F32, tag="stg", name=f"st{ft}{parity}{qh}"
                )
                nc.vector.tensor_copy(self.stg[:], self.pv[:])

            def normalize_items(self):
                """Per-chunk normalize closures (reciprocal + broadcast +
                multiply), sprinkled into later passes so the slow DVE
                reciprocal never blocks the critical DVE queue."""
                ft, qh, parity, q_base = self.ft, self.qh, self.parity, self.q_base
                stg = self.stg

                def norm_item(qc):
                    def run():
                        rc = rc_pool.tile(
                            [1, QCH], F32, tag="rc", name=f"rc{ft}{parity}{qh}{qc}"
                        )
                        nc.vector.reciprocal(
                            rc[:], stg[HD : HD + 1, qc * QCH : (qc + 1) * QCH]
                        )
                        bcs = bcs_pool.tile(
                            [HD, QCH], F32, tag="bcs", name=f"bc{ft}{parity}{qh}{qc}"
                        )
                        nc.gpsimd.partition_broadcast(bcs[:], rc[:])
                        with nc.allow_low_precision(reason="attn out cast"):
                            nc.vector.tensor_mul(
                                outP[ft][
                                    parity * HD : (parity + 1) * HD,
                                    q_base + qc * QCH : q_base + (qc + 1) * QCH,
                                ],
                                stg[:HD, qc * QCH : (qc + 1) * QCH],
                                bcs[:],
                            )

                    return run

                return [norm_item(qc) for qc in range(n_j)]

        pending_norm = []

        def attention_pass(ft, qh, parity, work):
            a = AttnQH(ft, qh, parity)
            stride = max(1, n_kt // max(1, len(work))) if work else 1
            a.emit_kts(range(n_kt), sprinkle=work, stride=stride)
            a.finish_stage1()
            return a.normalize_items()

        def y_work_items(qt_range, wo_t, ys_pool):
            items = []

            def y_item(qt):
                def run():
                    for no in range(n_no):
                        yp = ps_sc.tile([P, QCH], F32, tag="sc", name=f"yp{qt}_{no}")
                        for ft in range(n_ft):
                            nc.tensor.matmul(
                                yp[:],
                                outP[ft][:, qt * P : (qt + 1) * P],
                                wo_t[ft][:, no * QCH : (no + 1) * QCH],
                                start=(ft == 0),
                                stop=(ft == n_ft - 1),
                            )
                        ys = ys_pool.tile([P, QCH], F32, tag="ys", name=f"ys{qt}_{no}")
                        nc.vector.tensor_copy(ys[:], yp[:])
                        nc.sync.dma_start(
                            y[qt * P : (qt + 1) * P, no * QCH : (no + 1) * QCH], ys[:]
                        )

                return run

            for qt in qt_range:
                items.append(y_item(qt))
            return items

        # ---------------- emission ----------------
        if small:
            make_main_pools()
        qTp0, kTp0 = get_pair(0)
        att00 = None
        do_early = n_qh > 1 and n_ch >= 4 and small
        for ch in range(n_ch):
            items = upfront_chunk(ch, qTp0, kTp0, defer_v=do_early)
            if do_early:
                # interleave the deferred v-projections (and later chunks'
                # transposes) with the early (pair0, half0, headA) attention
                if ch == 0:
                    pend_items = items
                elif ch == 1:
                    att00 = AttnQH(0, 0, 0)
                    work = pend_items + items
                    att00.emit_kts(range(n_kt // 2), sprinkle=work, stride=1)
                    for it in work:
                        it()
                else:
                    lo = (n_kt // 4) * ch
                    att00.emit_kts(range(lo, lo + n_kt // 4), sprinkle=items,
                                   stride=1)
                    for it in items:
                        it()

        if not small:
            up_stack.close()
            make_main_pools()

        wo_t = ys_pool = None
        pending = []  # deferred normalize items from the previous pass
        for ft in range(n_ft):
            proj_items = proj_work_items(ft + 1) if ft + 1 < n_ft else []
            last = ft == n_ft - 1
            if last:
                # free the xT stream pool, load wout for the out-projection
                xTs_stack.close()
                wo_pool = top.enter_context(tc.tile_pool(name="wo", bufs=1))
                ys_pool = top.enter_context(tc.tile_pool(name="ys", bufs=3))
                wo_t = [
                    wo_pool.tile([P, DO], dt_o, tag=f"wo{ft2}", name=f"wo{ft2}")
                    for ft2 in range(n_ft)
                ]
                for ft2 in range(n_ft):
                    nc.gpsimd.dma_start(wo_t[ft2][:], wout[ft2 * P : (ft2 + 1) * P, :])
            rows_per_qh = n_st // n_qh
            # y rows become emittable one q-half late (after its normalizes)
            for qh in range(n_qh):
                passes = []
                if ft == 0 and qh == 0 and att00 is not None:
                    passes.append(att00)
                    passes.append(None)  # parity 1 fresh
                else:
                    passes.append(None)
                    passes.append(None)
                for parity in (0, 1):
                    work = pending + proj_items
                    pending, proj_items = [], []
                    if last and qh > 0 and parity == 1:
                        work += y_work_items(
                            range((qh - 1) * rows_per_qh, qh * rows_per_qh),
                            wo_t,
                            ys_pool,
                        )
                    a = passes[parity]
                    if a is None:
                        a = AttnQH(ft, qh, parity)
                        stride = max(1, n_kt // max(1, len(work))) if work else 1
                        a.emit_kts(range(n_kt), sprinkle=work, stride=stride)
                    a.finish_stage1()
                    for it in work:
                        it()
                    work.clear()
                    pending = a.normalize_items()
        # tail: last normalizes + last q-half's y rows
        for it in pending:
            it()
        for it in y_work_items(
            range((n_qh - 1) * rows_per_qh, n_qh * rows_per_qh), wo_t, ys_pool
        ):
            it()

    nc.compile()
    return nc


# problem sizes (hardcoded per contract)
B, S, D, H = 4, 2048, 1024, 16
DO = D
HN = H // 2  # heads per core
SCALE = (D // H) ** -0.5
N_CORES = 8

_NC_CACHE = None


def _get_nc():
    global _NC_CACHE
    if _NC_CACHE is None:
        _NC_CACHE = build_attention(S, D, HN, DO, SCALE)
    return _NC_CACHE


def make_in_maps(x, w_qkv, w_out):
    """Shard full inputs into the 8 per-core input maps."""
    in_maps = []
    for c in range(N_CORES):
        b = c // 2
        cs = (c % 2) * HN * HD
        ce = cs + HN * HD
        in_maps.append(
            {
                "x": np.ascontiguousarray(x[b]),
                "wq": np.ascontiguousarray(w_qkv[:, cs:ce]),
                "wk": np.ascontiguousarray(w_qkv[:, D + cs : D + ce]),
                "wv": np.ascontiguousarray(w_qkv[:, 2 * D + cs : 2 * D + ce]),
                "wout": np.ascontiguousarray(w_out[cs:ce, :]),
            }
        )
    return in_maps


def combine_outputs(results, b_out):
    """Sum the two per-batch partials and add the bias."""
    y = np.empty((B, S, DO), dtype=np.float32)
    for b in range(B):
        y[b] = results[2 * b]["y"] + results[2 * b + 1]["y"] + b_out[None, :]
    return y


def kernel(x, w_qkv, w_out, b_out):
    x = np.asarray(x, dtype=np.float32)
    w_qkv = np.asarray(w_qkv, dtype=np.float32)
    w_out = np.asarray(w_out, dtype=np.float32)
    b_out = np.asarray(b_out, dtype=np.float32)
    nc = _get_nc()
    in_maps = make_in_maps(x, w_qkv, w_out)
    res = bass_utils.run_bass_kernel_spmd(nc, in_maps, core_ids=list(range(N_CORES)))
    return combine_outputs(res.results, b_out)

